# revision 83
# baseline (speedup 1.0000x reference)
"""NeuralHMM forward log-likelihood on 8 Trainium2 NeuronCores.

Redesign v2 (pre-scaled operators, matmul-only chains):
  - Core k owns time slab [256k, 256k+256) for all B. Phase 1 computes, per
    128-step tile: transition MLP -> E = exp(logits) (bf16), row sums R,
    obs-model log-probs -> normalizer n(b,t) and ehat; then folds the whole
    per-step diagonal scale s_t = ehat_t * invR_{t+1} (chunk-final: ehat_t)
    INTO the stored transition kernel: Ehat_t[i,j] = E_t[i,j] * s_t[j]
    (one broadcast multiply over the big E tile), written to DRAM once.
  - Phase 2 is then matmul + PSUM->SBUF copy ONLY: chain state X (bf16,
    [128,64]) stacks two batch rows; the step lhsT is a block-diagonal
    [128,128] tile holding two chains' Ehat, so 8 matmuls + 8 copies per
    step (copies split DVE/Act) cover all 16 (b, 64-step-chunk) chains.
    Chunk init X = D(invR_lo) and the global-t=0 correction ride on tiny
    per-chain blends (alpha input).
  - Each core locally pre-multiplies its 4 chunk operators per b into one
    (PE transposes + matmuls after a DRAM bounce to partitions 0:64), so
    the AllGather ships only B operators per core (4x smaller); every core
    bulk-loads the 32 gathered operators (one DMA per core) and runs the
    8-step u-chain u <- Q^T u ([64,B] columns, renorm each step), then
    ll_b = log(u . ehat_0) + sum_t n(b,t) + sum ln(renorm) - log S.
Weight-only reshapes/transposes are precomputed on host in kernel().
"""

import math
import os
import sys

import numpy as np

BUILD_STAGE = int(os.environ.get("NHMM_STAGE", "3"))  # 1=p1, 2=+chains, 3=all
NOCC = os.environ.get("NHMM_NOCC", "0") == "1"        # skip collective (sim)
P2ONLY = os.environ.get("NHMM_P2ONLY", "0") == "1"    # timing sim: skip phase 1
REPEAT = int(os.environ.get("NHMM_REPEAT", "1"))      # body repeats (timing)

sys.path.insert(0, "/opt/trn_rl_repo")

import ml_dtypes  # noqa: E402

import concourse.bass as bass  # noqa: E402
import concourse.tile as tile  # noqa: E402
from concourse import bacc, mybir  # noqa: E402
from concourse.bass_utils import run_bass_kernel_spmd  # noqa: E402
from concourse.masks import make_identity  # noqa: E402

F32 = mybir.dt.float32
BF16 = mybir.dt.bfloat16
AF = mybir.ActivationFunctionType
AX = mybir.AxisListType
ALU = mybir.AluOpType

B, T, D = 4, 2048, 80
S, H, C = 64, 256, 128
NCORES = 8
SLAB = T // NCORES        # 256 timesteps per core
NT = 128                  # phase-1 tile width
NTILES = SLAB // NT       # 2 phase-1 tiles per (core, b)
CHUNK = int(os.environ.get("NHMM_CHUNK", "64"))   # chain chunk length
NSUB = SLAB // CHUNK      # chunks per (core, b)
NPAIR = 2 * NSUB          # chain pairs per core: (bpair, sub)
NCHAIN = 2 * NPAIR        # chains per core
SEG = {128: 32, 64: 16, 32: 16}[CHUNK]  # steps per eseg DMA
NSEG = CHUNK // SEG
CMAP = os.environ.get("NHMM_CMAP", "DDDDAAAA"[:NPAIR])
LSLICE = 512
NSL = (S * S) // LSLICE   # 8 logits slices per tile
GCHUNK = NCORES * NSUB    # 32 global chunks
NRN = NCORES              # combine renorm slots (renorm every u-chain step)

OFF_Q = 0                            # B per-core operators, 64x64 bf16 each
OFF_A0B = OFF_Q + B * S * S // 2         # [B, S] bf16 a0
OFF_NU = OFF_A0B + B * S // 2            # [B] f32
CONTRIB = OFF_NU + B


def build_program():
    nc = bacc.Bacc(
        "TRN2",
        target_bir_lowering=False,
        debug=False,
        enable_asserts=False,
        num_devices=NCORES,
    )

    def din(name, shape, dtype=F32):
        return nc.dram_tensor(name, list(shape), dtype, kind="ExternalInput").ap()

    io = {}
    io["obs"] = din("obs_slab", (B, SLAB, D))
    io["ctx"] = din("ctx_slab", (B, SLAB, C), BF16)
    io["alpha"] = din("alpha_pair", (128, NPAIR))
    io["tw1cT"] = din("tw1cT", (C, H), BF16)
    io["tb1p"] = din("tb1p", (H,))
    io["tw2T"] = din("tw2T", (H, H), BF16)
    io["tb2"] = din("tb2", (H,))
    io["tw3T"] = din("tw3T", (H, S * S), BF16)
    io["tb3"] = din("tb3_bf", (S * S,), BF16)
    io["fw1T"] = din("fw1T", (D, H), BF16)
    io["fb1"] = din("fb1_", (H,))
    io["fw2T"] = din("fw2T", (H, H), BF16)
    io["fb2"] = din("fb2_", (H,))
    io["mwT"] = din("mwT", (H, D), BF16)
    io["mb"] = din("mb_", (D,))
    io["lwT"] = din("lwT", (H, D), BF16)
    io["lb"] = din("lb_", (D,))
    io["lwsum"] = din("lwsum", (H,), BF16)
    io["L1"] = din("L1", (D, S), BF16)
    io["L2m"] = din("L2m", (D, S), BF16)
    io["L3"] = din("L3", (D, S), BF16)
    io["olv_bias"] = din("olv_bias", (S,))

    io["out"] = nc.dram_tensor("ll_out", [B], F32, kind="ExternalOutput").ap()
    io["Ebuf"] = nc.dram_tensor("Ebuf", [B, SLAB, S * S], BF16).ap()
    io["initd"] = nc.dram_tensor("initd", [B, NSUB, S], F32).ap()
    io["vd"] = nc.dram_tensor("vd", [B, S], F32).ap()
    io["lnd"] = nc.dram_tensor("lnd_bounce", [B, B], F32).ap()
    io["Mlocal"] = nc.dram_tensor("Mlocal", [NCHAIN, S, S], BF16).ap()
    io["contrib"] = nc.dram_tensor("contrib", [CONTRIB], F32).ap()
    io["gathered"] = nc.dram_tensor(
        "gathered", [NCORES * CONTRIB], F32, addr_space="Shared"
    ).ap()

    with tile.TileContext(nc) as tc:
        build_tile(tc, io)
    nc.compile()
    return nc


def build_tile(tc, io):
    nc = tc.nc
    dma = nc.sync
    with (
        tc.tile_pool(name="wts", bufs=1) as wts,
        tc.tile_pool(name="consts", bufs=1) as consts,
    ):
        ident = consts.tile([128, 128], F32)
        make_identity(nc, ident)
        identb = consts.tile([128, 128], BF16)
        nc.vector.tensor_copy(identb, ident)
        # stacked double identity [128, 64]: rows k, col k%64
        i2b = consts.tile([128, S], BF16)
        nc.vector.tensor_add(i2b, identb[:, 0:S], identb[:, S:2 * S])

        def load_w(tag, ap_dram, shape, dtype=F32):
            t = wts.tile(list(shape), dtype, tag=tag)
            dma.dma_start(out=t, in_=ap_dram)
            return t

        w = {}
        w["tw1cT"] = load_w("w1", io["tw1cT"], (C, H), BF16)
        w["tw2T"] = [load_w(f"w2{k}", io["tw2T"][k * 128:(k + 1) * 128, :],
                            (128, H), BF16) for k in range(2)]
        w["tw3T"] = [load_w(f"w3{k}", io["tw3T"][k * 128:(k + 1) * 128, :],
                            (128, S * S), BF16) for k in range(2)]
        w["tb3"] = load_w("b3", io["tb3"][None, :], (1, S * S), BF16)
        w["fw1T"] = load_w("g1", io["fw1T"], (D, H), BF16)
        w["fw2T"] = [load_w(f"g2{k}", io["fw2T"][k * 128:(k + 1) * 128, :],
                            (128, H), BF16) for k in range(2)]
        w["mwT"] = [load_w(f"gm{k}", io["mwT"][k * 128:(k + 1) * 128, :],
                           (128, D), BF16) for k in range(2)]
        w["lwT"] = [load_w(f"gl{k}", io["lwT"][k * 128:(k + 1) * 128, :],
                           (128, D), BF16) for k in range(2)]
        w["lwsum"] = [load_w(f"ls{k}", io["lwsum"][k * 128:(k + 1) * 128, None],
                             (128, 1), BF16) for k in range(2)]
        w["L1"] = load_w("L1", io["L1"], (D, S), BF16)
        w["L2m"] = load_w("L2m", io["L2m"], (D, S), BF16)
        w["L3"] = load_w("L3", io["L3"], (D, S), BF16)
        for nm in ("tb1p", "tb2", "fb1", "fb2"):
            w[nm] = [load_w(f"{nm}{k}", io[nm][k * 128:(k + 1) * 128, None],
                            (128, 1)) for k in range(2)]
        w["mb"] = load_w("mb", io["mb"][:, None], (D, 1))
        w["lb"] = load_w("lb", io["lb"][:, None], (D, 1))
        w["olvb"] = load_w("olvb", io["olv_bias"][:, None], (S, 1))
        w["alpha"] = load_w("alpha", io["alpha"], (128, NPAIR))

        ones_bt = consts.tile([1, NT], BF16)        # lhsT for tb3 rank-1
        nc.vector.memset(ones_bt, 1.0)
        pones_bf = consts.tile([1, S], BF16)        # +1 lhsT (sum_blv bcast)
        nc.vector.memset(pones_bf, 1.0)
        pones128 = consts.tile([1, 128], F32)       # rbc broadcast (combine)
        nc.vector.memset(pones128, 1.0)
        nones_row = consts.tile([1, S], F32)        # -1 lhsT (mean bcast)
        nc.vector.memset(nones_row, -1.0)
        invS_col = consts.tile([S, 1], F32)
        nc.vector.memset(invS_col, 1.0 / S)
        ones128 = consts.tile([128, 1], BF16)       # mass lhsT (combine)
        nc.vector.memset(ones128, 1.0)
        w.update(ones_bt=ones_bt, pones_bf=pones_bf, pones128=pones128,
                 nones_row=nones_row, invS_col=invS_col, ones128=ones128,
                 ident=ident, identb=identb, i2b=i2b)

        nu_sb = consts.tile([1, B], F32)
        a0_sb = consts.tile([S, B], F32)
        sp0_sb = consts.tile([S, B], F32)
        initv_sb = consts.tile([S, B, NSUB], F32)
        nslots_sb = consts.tile([1, B, NSUB], F32)

        # eseg tiles live across both phases; zero the off-diag blocks once,
        # before phase 1, so the memsets overlap compute
        with tc.tile_pool(name="p2e", bufs=1) as p2e:
            eseg = []
            for p in range(NPAIR):
                row = []
                for s in range(2):
                    et = p2e.tile([128, SEG, 128], BF16, tag=f"es{p}_{s}")
                    nc.gpsimd.memset(et, 0.0)
                    row.append(et)
                eseg.append(row)
            for rep in range(REPEAT):
                build_main(tc, io, w, dma, nu_sb, a0_sb, sp0_sb, initv_sb,
                           nslots_sb, eseg, rep)


def build_main(tc, io, w, dma, nu_sb, a0_sb, sp0_sb, initv_sb, nslots_sb,
               eseg, rep=0):
    nc = tc.nc
    contrib = io["contrib"]
    if True:
        # ================= PHASE 1 =================
        with (
            tc.tile_pool(name="p1", bufs=3) as p1,
            tc.tile_pool(name="p1es", bufs=2) as p1es,
            tc.tile_pool(name="pbig", bufs=3, space="PSUM") as pbig,
            tc.tile_pool(name="plg", bufs=2, space="PSUM") as plg,
            tc.tile_pool(name="psm", bufs=3, space="PSUM") as psm,
        ):
            if P2ONLY:
                for t in (a0_sb, sp0_sb, initv_sb, nslots_sb):
                    nc.vector.memset(t, 1.0)
            for b in range(B):
                for hh in range(NTILES):
                    if not P2ONLY:
                        phase1_tile(nc, b, hh, dma, p1, p1es, pbig, plg, psm,
                                    w, io, a0_sb, sp0_sb, initv_sb, nslots_sb)
            for b in range(B):
                nc.vector.reduce_sum(nu_sb[:, b:b + 1], nslots_sb[:, b, :],
                                     axis=AX.X)
            # small DRAM bounces for phase-2 init (cross-partition placement)
            dma.dma_start(
                out=io["initd"].rearrange("b n s -> s (b n)"),
                in_=initv_sb.rearrange("s b n -> s (b n)"),
            )
            vtmp = p1.tile([S, B], F32, tag="vtmp")
            nc.vector.reciprocal(vtmp, a0_sb)
            vtmp2 = p1.tile([S, B], F32, tag="vtmp2")
            nc.vector.tensor_mul(vtmp2, vtmp, sp0_sb)
            dma.dma_start(out=io["vd"].rearrange("b s -> s b"), in_=vtmp2)
            contrib = io["contrib"]
            dma.dma_start(out=contrib[OFF_NU:OFF_NU + B][None, :], in_=nu_sb)
            a0bf = p1.tile([S, B], BF16, tag="a0bf")
            nc.vector.tensor_copy(a0bf, a0_sb)
            dma.dma_start(
                out=contrib[OFF_A0B:OFF_A0B + B * S // 2].bitcast(BF16)
                .rearrange("(b s2) -> s2 b", s2=S),
                in_=a0bf,
            )

        if BUILD_STAGE < 2:
            return
        # ================= PHASE 2 =================
        with (
            tc.tile_pool(name="p2s", bufs=2) as p2s,
            tc.tile_pool(name="p2x", bufs=2) as p2x,
            tc.tile_pool(name="p2ps", bufs=1, space="PSUM") as p2ps,
        ):
            xfin = run_chains(nc, dma, eseg, p2s, p2x, p2ps, w, io)
            # chunk operators -> local DRAM (bounce to get all blocks onto
            # partitions 0:64 for the local pre-combine)
            for p in range(NPAIR):
                for h in range(2):
                    c = p * 2 + h
                    dma.dma_start(
                        out=io["Mlocal"][c],
                        in_=xfin[p][h * S:(h + 1) * S, :],
                    )
            # local pre-combine: per b, multiply this core's 4 chunk
            # operators (ascending sub) into one; ship only those 4.
            mload = p2s.tile([S, NCHAIN, S], BF16, tag="mload")
            dma.dma_start(out=mload,
                          in_=io["Mlocal"].rearrange("c a m -> a c m"))

            def prodmm(left, right, pj):
                # left @ right via lhsT = left^T (PE transpose, base 0)
                tps = p2ps.tile([S, S], BF16, tag=f"ps{pj}")
                nc.tensor.transpose(tps, left, w["identb"][:S, :S])
                tsb = p2x.tile([S, S], BF16, tag=f"tsb{pj}")
                nc.vector.tensor_copy(tsb, tps)
                pps = p2ps.tile([S, S], F32, tag=f"ps{pj + 1}")
                nc.tensor.matmul(pps, tsb, right)
                osb = p2x.tile([S, S], BF16, tag=f"osb{pj}")
                nc.vector.tensor_copy(osb, pps)
                return osb

            qreg = contrib[OFF_Q:OFF_Q + B * S * S // 2].bitcast(BF16)
            for b in range(B):
                bp, h = b // 2, b % 2

                def opv(sub):
                    return mload[:, bp * 2 * NSUB + sub * 2 + h, :]

                p1r = prodmm(opv(1), opv(0), (2 * b) % 6)
                p2r = prodmm(opv(3), opv(2), (2 * b + 1) % 6)
                qb = prodmm(p2r, p1r, (2 * b) % 6)
                dma.dma_start(
                    out=qreg[b * S * S:(b + 1) * S * S]
                    .rearrange("(a m) -> a m", a=S),
                    in_=qb,
                )
        if BUILD_STAGE < 3:
            return
        if not NOCC:
            with tc.tile_critical():
                with nc.semaphore(f"ccsem{rep}") as ccsem:
                    nc.gpsimd.collective_compute(
                        "AllGather",
                        ALU.bypass,
                        replica_groups=[list(range(NCORES))],
                        ins=[contrib],
                        outs=[io["gathered"]],
                    ).then_inc(ccsem, 1)
                    nc.gpsimd.wait_ge(ccsem, 1)

        # ================= COMBINE =================
        with (
            tc.tile_pool(name="cmb", bufs=2) as cmb,
            tc.tile_pool(name="cmbps", bufs=1, space="PSUM") as cmbps,
        ):
            combine(nc, dma, cmb, cmbps, w, io)


def phase1_tile(nc, b, hh, dma, p1, p1es, pbig, plg, psm, w, io,
                a0_sb, sp0_sb, initv_sb, nslots_sb):
    nt = NT
    t0 = hh * NT
    ident = w["ident"]

    ctx_t = p1.tile([nt, C], BF16, tag="ctxt")
    dma.dma_start(out=ctx_t, in_=io["ctx"][b, t0:t0 + nt, :])
    obs_t = p1.tile([nt, D], F32, tag="obst")
    dma.dma_start(out=obs_t, in_=io["obs"][b, t0:t0 + nt, :])

    ctxT_ps = pbig.tile([C, nt], BF16, tag="pp")
    nc.tensor.transpose(ctxT_ps, ctx_t, w["identb"])
    ctxT = p1.tile([C, nt], BF16, tag="ctxT")
    nc.vector.tensor_copy(ctxT, ctxT_ps)

    obsT_ps = psm.tile([D, nt], F32, tag="sm")
    nc.tensor.transpose(obsT_ps, obs_t, ident)
    obsT = p1.tile([D, nt], F32, tag="obsT")
    nc.vector.tensor_copy(obsT, obsT_ps)
    obsT_bf = p1.tile([D, nt], BF16, tag="obsTb")
    nc.scalar.copy(obsT_bf, obsT_ps)

    # transition MLP (feature-on-partition, bf16)
    h1 = []
    for m in range(2):
        ps = pbig.tile([128, nt], F32, tag="pp")
        nc.tensor.matmul(ps, w["tw1cT"][:, m * 128:(m + 1) * 128], ctxT)
        sb = p1.tile([128, nt], BF16, tag=f"h1_{m}")
        nc.scalar.activation(sb, ps, AF.Relu, bias=w["tb1p"][m], scale=1.0)
        h1.append(sb)
    h2 = []
    for m in range(2):
        ps = pbig.tile([128, nt], F32, tag="pp")
        for k in range(2):
            nc.tensor.matmul(ps, w["tw2T"][k][:, m * 128:(m + 1) * 128], h1[k],
                             start=(k == 0), stop=(k == 1))
        sb = p1.tile([128, nt], BF16, tag=f"h2_{m}")
        nc.scalar.activation(sb, ps, AF.Relu, bias=w["tb2"][m], scale=1.0)
        h2.append(sb)

    # logits slices -> exp (bf16) into the big E tile; R accumulation
    E_all = p1es.tile([nt, S * S], BF16, tag="eall")
    R_sb = p1.tile([nt, S], F32, tag="Rsb")
    ni = LSLICE // S
    for sl in range(NSL):
        ps = plg.tile([nt, LSLICE], F32, tag="lgp")
        for k in range(2):
            nc.tensor.matmul(ps, h2[k],
                             w["tw3T"][k][:, sl * LSLICE:(sl + 1) * LSLICE],
                             start=(k == 0), stop=False)
        nc.tensor.matmul(ps, w["ones_bt"],
                         w["tb3"][:, sl * LSLICE:(sl + 1) * LSLICE],
                         start=False, stop=True)
        esl = E_all[:, sl * LSLICE:(sl + 1) * LSLICE]
        nc.scalar.activation(esl, ps, AF.Exp)
        nc.vector.reduce_sum(
            R_sb[:, sl * ni:(sl + 1) * ni],
            esl.rearrange("p (i j) -> p i j", j=S),
            axis=AX.X,
        )

    # observation model (bf16 MLP)
    f1 = []
    for m in range(2):
        ps = pbig.tile([128, nt], F32, tag="pp")
        nc.tensor.matmul(ps, w["fw1T"][:, m * 128:(m + 1) * 128], obsT_bf)
        sb = p1.tile([128, nt], BF16, tag=f"f1_{m}")
        nc.scalar.activation(sb, ps, AF.Relu, bias=w["fb1"][m], scale=1.0)
        f1.append(sb)
    f2 = []
    for m in range(2):
        ps = pbig.tile([128, nt], F32, tag="pp")
        for k in range(2):
            nc.tensor.matmul(ps, w["fw2T"][k][:, m * 128:(m + 1) * 128], f1[k],
                             start=(k == 0), stop=(k == 1))
        sb = p1.tile([128, nt], BF16, tag=f"f2_{m}")
        nc.scalar.activation(sb, ps, AF.Relu, bias=w["fb2"][m], scale=1.0)
        f2.append(sb)

    bm_ps = psm.tile([D, nt], F32, tag="sm")
    for k in range(2):
        nc.tensor.matmul(bm_ps, w["mwT"][k], f2[k], start=(k == 0), stop=(k == 1))
    bm = p1.tile([D, nt], F32, tag="bm")
    nc.scalar.activation(bm, bm_ps, AF.Identity, bias=w["mb"], scale=1.0)

    blv_ps = psm.tile([D, nt], F32, tag="sm")
    for k in range(2):
        nc.tensor.matmul(blv_ps, w["lwT"][k], f2[k], start=(k == 0), stop=(k == 1))
    blv = p1.tile([D, nt], F32, tag="blv")
    nc.scalar.activation(blv, blv_ps, AF.Identity, bias=w["lb"], scale=1.0)

    r_ = p1.tile([D, nt], F32, tag="r_")
    nc.vector.tensor_sub(r_, obsT, bm)
    e_ = p1.tile([D, nt], F32, tag="e_")
    nc.scalar.activation(e_, blv, AF.Exp, scale=-1.0)
    rr = p1.tile([D, nt], F32, tag="rr")
    nc.vector.tensor_mul(rr, r_, r_)
    A_ = p1.tile([D, nt], BF16, tag="A_")
    nc.vector.tensor_mul(A_, rr, e_)
    Bm_ = p1.tile([D, nt], BF16, tag="Bm_")
    nc.vector.tensor_mul(Bm_, r_, e_)
    e_bf = p1.tile([D, nt], BF16, tag="ebf")
    nc.vector.tensor_copy(e_bf, e_)

    sb_ps = psm.tile([1, nt], F32, tag="sm")
    for k in range(2):
        nc.tensor.matmul(sb_ps, w["lwsum"][k], f2[k], start=(k == 0),
                         stop=(k == 1))
    sblv = p1.tile([1, nt], BF16, tag="sblv")
    nc.scalar.copy(sblv, sb_ps)

    q_ps = psm.tile([S, nt], F32, tag="sm")
    nc.tensor.matmul(q_ps, w["L1"], A_, start=True, stop=False)
    nc.tensor.matmul(q_ps, w["L2m"], Bm_, start=False, stop=False)
    nc.tensor.matmul(q_ps, w["L3"], e_bf, start=False, stop=False)
    nc.tensor.matmul(q_ps, w["pones_bf"], sblv, start=False, stop=True)

    lp0 = p1.tile([S, nt], F32, tag="lp0")
    nc.scalar.activation(lp0, q_ps, AF.Identity, bias=w["olvb"], scale=-0.5)

    n_ps = psm.tile([1, nt], F32, tag="sm")
    nc.tensor.matmul(n_ps, w["invS_col"], lp0)
    n_sb = p1.tile([1, nt], F32, tag="nsb")
    nc.scalar.copy(n_sb, n_ps)
    d_ps = psm.tile([S, nt], F32, tag="sm")
    nc.tensor.matmul(d_ps, w["nones_row"], n_sb)      # = -n broadcast
    dd = p1.tile([S, nt], F32, tag="dd")
    nc.vector.tensor_add(dd, lp0, d_ps)
    ehat = p1.tile([S, nt], F32, tag="ehat")
    nc.scalar.activation(ehat, dd, AF.Exp)

    # invR and the per-step scale s (transposed form first)
    RT_ps = psm.tile([S, nt], F32, tag="sm")
    nc.tensor.transpose(RT_ps, R_sb, ident)
    invR = p1.tile([S, nt], F32, tag="invR")
    nc.vector.reciprocal(invR, RT_ps)

    shT = p1.tile([S, nt], F32, tag="shT")
    for cc in range(nt // CHUNK):
        lo = cc * CHUNK
        nc.vector.tensor_mul(shT[:, lo:lo + CHUNK - 1],
                             ehat[:, lo:lo + CHUNK - 1],
                             invR[:, lo + 1:lo + CHUNK])
        nc.vector.tensor_copy(shT[:, lo + CHUNK - 1:lo + CHUNK],
                              ehat[:, lo + CHUNK - 1:lo + CHUNK])

    sh_ps = psm.tile([nt, S], F32, tag="sm")
    nc.tensor.transpose(sh_ps, shT, ident[:S, :S])
    shat = p1.tile([nt, S], BF16, tag="shat")
    nc.vector.tensor_copy(shat, sh_ps)

    # pre-scale the big E tile along j, then single DMA out. SBUF-only
    # elementwise, so part of it can ride the (otherwise idle) GPSIMD.
    ev3 = E_all.rearrange("p (i j) -> p i j", j=S)
    shb = shat[:, None, :]
    psplit = os.environ.get("NHMM_PSPLIT", "DDPP")
    for q4 in range(4):
        blk = 16
        eng = nc.gpsimd if psplit[q4] == "P" else nc.vector
        eng.tensor_mul(
            ev3[:, q4 * blk:(q4 + 1) * blk, :],
            ev3[:, q4 * blk:(q4 + 1) * blk, :],
            shb.broadcast_to([nt, blk, S]),
        )
    ebq = os.environ.get("NHMM_EBUFQ", "pool")
    eng = {"sp": nc.sync, "act": nc.scalar, "pool": nc.gpsimd}[ebq]
    eng.dma_start(out=io["Ebuf"][b, t0:t0 + nt, :], in_=E_all)

    # persists for phase 2 / combine
    for cc in range(nt // CHUNK):
        sub = hh * (nt // CHUNK) + cc
        nc.vector.tensor_copy(initv_sb[:, b, sub:sub + 1],
                              invR[:, cc * CHUNK:cc * CHUNK + 1])
    if hh == 0:
        nc.vector.tensor_copy(a0_sb[:, b:b + 1], ehat[:, 0:1])
        nc.vector.tensor_copy(sp0_sb[:, b:b + 1], shT[:, 0:1])
    nc.vector.reduce_sum(nslots_sb[:, b, hh:hh + 1], n_sb, axis=AX.X)


def run_chains(nc, dma, eseg, p2s, p2x, p2ps, w, io):
    """8 pair-chains; pair p = (bpair=p//NSUB, sub=p%NSUB) packs chains
    b=2*bpair and b=2*bpair+1 block-diagonally."""
    alpha = w["alpha"]
    Ebuf = io["Ebuf"]
    ev = [Ebuf[b].rearrange("t (i j) -> i t j", i=S) for b in range(B)]

    def issue_seg(p, sgi):
        bp, sub = p // NSUB, p % NSUB
        lt0 = sub * CHUNK + sgi * SEG
        et = eseg[p][sgi % 2]
        for h in range(2):
            b = 2 * bp + h
            dma.dma_start(
                out=et[h * S:(h + 1) * S, :, h * S:(h + 1) * S],
                in_=ev[b][:, lt0:lt0 + SEG, :],
            )

    for p in range(NPAIR):
        issue_seg(p, 0)
        issue_seg(p, 1)

    # X init: stacked diag(1 + alpha*(invR_lo - 1))
    xall = []
    for p in range(NPAIR):
        bp, sub = p // NSUB, p % NSUB
        irp = p2s.tile([128, 1], F32, tag=f"irp{p}")
        for h in range(2):
            nc.scalar.dma_start(
                out=irp[h * S:(h + 1) * S, :],
                in_=io["initd"][2 * bp + h, sub, :][:, None],
            )
        t1 = p2s.tile([128, 1], F32, tag="t1")
        nc.vector.tensor_scalar_add(t1, irp, -1.0)
        t2 = p2s.tile([128, 1], F32, tag="t2")
        nc.vector.tensor_mul(t2, t1, alpha[:, p:p + 1])
        t3 = p2s.tile([128, 1], F32, tag="t3")
        nc.vector.tensor_scalar_add(t3, t2, 1.0)
        xp = p2x.tile([128, S], BF16, tag=f"x{p}")
        nc.vector.tensor_scalar_mul(xp, w["i2b"], t3)
        xall.append(xp)

    # step-0 blend for sub=0 pairs: et0 <- alpha*et0 + (1-alpha)*D(v)
    for p in range(NPAIR):
        if p % NSUB != 0:
            continue
        bp = p // NSUB
        vp = p2s.tile([128, 1], F32, tag=f"vp{p}")
        for h in range(2):
            nc.scalar.dma_start(
                out=vp[h * S:(h + 1) * S, :],
                in_=io["vd"][2 * bp + h, :][:, None],
            )
        w1 = p2s.tile([128, 1], F32, tag="w1")
        nc.vector.tensor_scalar(w1, alpha[:, p:p + 1], -1.0, 1.0,
                                op0=ALU.mult, op1=ALU.add)
        w2 = p2s.tile([128, 1], F32, tag="w2")
        nc.vector.tensor_mul(w2, w1, vp)
        dmat = p2s.tile([128, 128], BF16, tag="dmat")
        nc.vector.tensor_scalar_mul(dmat, w["identb"], w2)
        et0 = eseg[p][0][:, 0, :]
        tmp_e = p2s.tile([128, 128], BF16, tag="tmpe")
        nc.vector.tensor_scalar_mul(tmp_e, et0, alpha[:, p:p + 1])
        nc.vector.tensor_add(et0, tmp_e, dmat)

    # copy-engine split per NHMM_CMAP (D=DVE, A=Act)
    def copy_x(p, new_x, ps):
        if CMAP[p % len(CMAP)] == "D":
            nc.vector.tensor_copy(new_x, ps)
        else:
            nc.scalar.copy(new_x, ps)

    for k in range(CHUNK):
        sgi, tt = k // SEG, k % SEG
        pss = []
        for p in range(NPAIR):
            ps = p2ps.tile([128, S], F32, tag=f"ps{p}")
            nc.tensor.matmul(ps, eseg[p][sgi % 2][:, tt, :], xall[p])
            pss.append(ps)
        for p in range(NPAIR):
            new_x = p2x.tile([128, S], BF16, tag=f"x{p}")
            copy_x(p, new_x, pss[p])
            xall[p] = new_x
        if tt == SEG - 1 and sgi + 2 < NSEG:
            for p in range(NPAIR):
                issue_seg(p, sgi + 2)

    return xall


def combine(nc, dma, cmb, cmbps, w, io):
    if NOCC:
        def gsl(core, lo, n):
            return io["contrib"][lo:lo + n]
    else:
        g2 = io["gathered"].rearrange("(k f) -> k f", k=NCORES)

        def gsl(core, lo, n):
            return g2[core, lo:lo + n]

    # Bulk-load the per-core pre-combined operators: one [64, B, 64] tile
    # per core (one DMA each); everything sits on partitions 0:64.
    mstore = {}
    for core in range(NCORES - 1, -1, -1):
        mv = (gsl(core, OFF_Q, B * S * S // 2).bitcast(BF16)
              .rearrange("(b a m) -> a b m", b=B, a=S))
        mt = cmb.tile([S, B, S], BF16, tag=f"ms_{core}")
        eng = dma if core % 2 == 0 else nc.scalar
        eng.dma_start(out=mt, in_=mv)
        mstore[core] = mt

    # u: [64, B] — one column per batch element
    u = cmb.tile([S, B], BF16, tag="u")
    nc.vector.memset(u, 1.0)
    mslots = cmb.tile([1, B, NRN], F32, tag="mslots")
    nc.vector.memset(mslots, 1.0)

    for step, core in enumerate(range(NCORES - 1, -1, -1)):
        mt = mstore[core]
        up = cmbps.tile([S, B], F32, tag="up")
        for b in range(B):
            nc.tensor.matmul(up[:, b:b + 1], mt[:, b, :], u[:, b:b + 1])
        un = cmb.tile([S, B], BF16, tag="u")
        nc.vector.tensor_copy(un, up)
        u = un
        # renorm every step: per-core operators span 256 timesteps, so
        # column masses can reach ~1e12 per application
        ms = cmbps.tile([1, B], F32, tag="msp")
        nc.tensor.matmul(ms, w["ones128"][0:S, :], un)
        nc.scalar.copy(mslots[:, :, step], ms)
        minv = cmb.tile([1, B], F32, tag="mi")
        nc.vector.reciprocal(minv, ms)
        rbc = cmbps.tile([S, B], F32, tag="rb")
        nc.tensor.matmul(rbc, w["pones128"][:, 0:S], minv)
        u2 = cmb.tile([S, B], BF16, tag="u")
        nc.vector.tensor_mul(u2, un, rbc)
        u = u2

    # dots with a0, logs, nu sums
    a0p = cmb.tile([S, B], BF16, tag="a0p")
    a0bit = gsl(0, OFF_A0B, B * S // 2).bitcast(BF16)
    for b in range(B):
        dma.dma_start(out=a0p[:, b:b + 1],
                      in_=a0bit[b * S:(b + 1) * S][:, None])
    dot = cmbps.tile([B, B], F32, tag="dot")
    nc.tensor.matmul(dot, u, a0p)
    lnd = cmb.tile([B, B], F32, tag="lnd")
    nc.scalar.activation(lnd, dot, AF.Ln)
    dma.dma_start(out=io["lnd"], in_=lnd)

    # bounce the diagonal entries back onto partition 0: [1, 4] = b order
    lnrow = cmb.tile([1, B], F32, tag="lnrow")
    for b in range(B):
        dma.dma_start(out=lnrow[:, b:b + 1],
                      in_=io["lnd"][b, b:b + 1][None, :])

    lnms = cmb.tile([1, B, NRN], F32, tag="lnms")
    nc.scalar.activation(lnms, mslots, AF.Ln)
    lnm_sb = cmb.tile([1, B], F32, tag="lnm")
    for b in range(B):
        nc.vector.reduce_sum(lnm_sb[:, b:b + 1], lnms[:, b, :], axis=AX.X)

    nurow = cmb.tile([1, B, NCORES], F32, tag="nur")
    for b in range(B):
        if NOCC:
            for kk in range(NCORES):
                dma.dma_start(out=nurow[:, b, kk:kk + 1],
                              in_=io["contrib"][OFF_NU + b:OFF_NU + b + 1][None, :])
        else:
            g2 = io["gathered"].rearrange("(k f) -> k f", k=NCORES)
            dma.dma_start(out=nurow[:, b, :], in_=g2[:, OFF_NU + b][None, :])
    nusum = cmb.tile([1, B], F32, tag="nus")
    for b in range(B):
        nc.vector.reduce_sum(nusum[:, b:b + 1], nurow[:, b, :], axis=AX.X)

    acc1 = cmb.tile([1, B], F32, tag="acc1")
    nc.vector.tensor_add(acc1, lnrow, lnm_sb)
    acc2 = cmb.tile([1, B], F32, tag="acc2")
    nc.vector.tensor_add(acc2, acc1, nusum)
    out_row = cmb.tile([1, B], F32, tag="outrow")
    nc.vector.tensor_scalar_add(out_row, acc2, -math.log(S))
    dma.dma_start(out=io["out"][None, :], in_=out_row)


# ======================================================================
# host side
# ======================================================================
_PROGRAM_CACHE = {}


def _get_program():
    if "nc" not in _PROGRAM_CACHE:
        _PROGRAM_CACHE["nc"] = build_program()
    return _PROGRAM_CACHE["nc"]


def host_prep(inp):
    f32 = np.float32
    bf = ml_dtypes.bfloat16
    p = {}
    tw1 = np.asarray(inp["tw1"], f32)
    p["tw1cT"] = np.ascontiguousarray(tw1[:, :C].T).astype(bf)
    p["tb1p"] = (np.asarray(inp["tb1"], f32) + tw1[:, C:].sum(1) / S).astype(f32)
    p["tw2T"] = np.ascontiguousarray(np.asarray(inp["tw2"], f32).T).astype(bf)
    p["tb2"] = np.asarray(inp["tb2"], f32)
    p["tw3T"] = np.ascontiguousarray(np.asarray(inp["tw3"], f32).T).astype(bf)
    p["tb3_bf"] = np.asarray(inp["tb3"], f32).astype(bf)
    p["fw1T"] = np.ascontiguousarray(np.asarray(inp["fw1"], f32).T).astype(bf)
    p["fb1_"] = np.asarray(inp["fb1"], f32)
    p["fw2T"] = np.ascontiguousarray(np.asarray(inp["fw2"], f32).T).astype(bf)
    p["fb2_"] = np.asarray(inp["fb2"], f32)
    lw = np.asarray(inp["lw"], f32)
    p["mwT"] = np.ascontiguousarray(np.asarray(inp["mw"], f32).T).astype(bf)
    p["mb_"] = np.asarray(inp["mb"], f32)
    p["lwT"] = np.ascontiguousarray(lw.T).astype(bf)
    p["lb_"] = np.asarray(inp["lb"], f32)
    p["lwsum"] = lw.sum(0).astype(bf)
    se = np.asarray(inp["state_emb"], f32)
    off_mean = se @ np.asarray(inp["mw"], f32).T
    off_lv = se @ lw.T
    E1 = np.exp(-off_lv)
    p["L1"] = np.ascontiguousarray(E1.T).astype(bf)
    p["L2m"] = np.ascontiguousarray((-2.0 * off_mean * E1).T).astype(bf)
    p["L3"] = np.ascontiguousarray((off_mean**2 * E1).T).astype(bf)
    p["olv_bias"] = (
        -0.5 * (D * math.log(2.0 * math.pi) + off_lv.sum(1))
        - 0.5 * np.asarray(inp["lb"], f32).sum()
    ).astype(f32)
    return p


def make_in_maps(inp, prepped=None):
    p = prepped if prepped is not None else host_prep(inp)
    obs = np.asarray(inp["observations"], np.float32)
    ctx = np.asarray(inp["context"], np.float32).astype(ml_dtypes.bfloat16)
    in_maps = []
    for k in range(NCORES):
        t0, t1 = SLAB * k, SLAB * (k + 1)
        alpha = np.ones((128, NPAIR), np.float32)
        if k == 0:
            alpha[:, 0] = 0.0   # sub-0 pairs: global t=0 correction
            alpha[:, NSUB] = 0.0
        m = {
            "obs_slab": np.ascontiguousarray(obs[:, t0:t1, :]),
            "ctx_slab": np.ascontiguousarray(ctx[:, t0:t1, :]),
            "alpha_pair": alpha,
        }
        m.update(p)
        in_maps.append(m)
    return in_maps


def kernel(**inputs):
    nc = _get_program()
    in_maps = make_in_maps(inputs)
    res = run_bass_kernel_spmd(nc, in_maps, core_ids=list(range(NCORES)))
    return np.asarray(res.results[0]["ll_out"], np.float32)


if __name__ == "__main__":
    sys.path.insert(0, "/root/problem")
    import jax

    with jax.default_device(jax.devices("cpu")[0]):
        import reference

        inp = {k: np.asarray(v) for k, v in reference.setup_inputs().items()}
        expected = np.asarray(reference.reference(**inp))
    got = kernel(**inp)
    print("expected:", expected)
    print("kernel:  ", got)
    rel = np.abs(got - expected) / np.abs(expected)
    print(f"rel: {rel.max():.3e}")


# revision 86
# speedup vs baseline: 1.0222x; 1.0222x over previous
"""NeuralHMM forward log-likelihood on 8 Trainium2 NeuronCores.

Redesign v2 (pre-scaled operators, matmul-only chains):
  - Core k owns time slab [256k, 256k+256) for all B. Phase 1 computes, per
    128-step tile: transition MLP -> E = exp(logits) (bf16), row sums R,
    obs-model log-probs -> normalizer n(b,t) and ehat; then folds the whole
    per-step diagonal scale s_t = ehat_t * invR_{t+1} (chunk-final: ehat_t)
    INTO the stored transition kernel: Ehat_t[i,j] = E_t[i,j] * s_t[j]
    (one broadcast multiply over the big E tile), written to DRAM once.
  - Phase 2 is then matmul + PSUM->SBUF copy ONLY: chain state X (bf16,
    [128,64]) stacks two batch rows; the step lhsT is a block-diagonal
    [128,128] tile holding two chains' Ehat, so 8 matmuls + 8 copies per
    step (copies split DVE/Act) cover all 16 (b, 64-step-chunk) chains.
    Chunk init X = D(invR_lo) and the global-t=0 correction ride on tiny
    per-chain blends (alpha input).
  - Each core locally pre-multiplies its 4 chunk operators per b into one
    (PE transposes + matmuls after a DRAM bounce to partitions 0:64), so
    the AllGather ships only B operators per core (4x smaller); every core
    bulk-loads the 32 gathered operators (one DMA per core) and runs the
    8-step u-chain u <- Q^T u ([64,B] columns, renorm each step), then
    ll_b = log(u . ehat_0) + sum_t n(b,t) + sum ln(renorm) - log S.
Weight-only reshapes/transposes are precomputed on host in kernel().
"""

import math
import os
import sys

import numpy as np

BUILD_STAGE = int(os.environ.get("NHMM_STAGE", "3"))  # 1=p1, 2=+chains, 3=all
NOCC = os.environ.get("NHMM_NOCC", "0") == "1"        # skip collective (sim)
P2ONLY = os.environ.get("NHMM_P2ONLY", "0") == "1"    # timing sim: skip phase 1
REPEAT = int(os.environ.get("NHMM_REPEAT", "1"))      # body repeats (timing)

sys.path.insert(0, "/opt/trn_rl_repo")

import ml_dtypes  # noqa: E402

import concourse.bass as bass  # noqa: E402
import concourse.tile as tile  # noqa: E402
from concourse import bacc, mybir  # noqa: E402
from concourse.bass_utils import run_bass_kernel_spmd  # noqa: E402
from concourse.masks import make_identity  # noqa: E402

F32 = mybir.dt.float32
BF16 = mybir.dt.bfloat16
AF = mybir.ActivationFunctionType
AX = mybir.AxisListType
ALU = mybir.AluOpType

B, T, D = 4, 2048, 80
S, H, C = 64, 256, 128
NCORES = 8
SLAB = T // NCORES        # 256 timesteps per core
NT = 128                  # phase-1 tile width
NTILES = SLAB // NT       # 2 phase-1 tiles per (core, b)
CHUNK = int(os.environ.get("NHMM_CHUNK", "64"))   # chain chunk length
NSUB = SLAB // CHUNK      # chunks per (core, b)
NPAIR = 2 * NSUB          # chain pairs per core: (bpair, sub)
NCHAIN = 2 * NPAIR        # chains per core
SEG = {128: 32, 64: 16, 32: 16}[CHUNK]  # steps per eseg DMA
NSEG = CHUNK // SEG
CMAP = os.environ.get("NHMM_CMAP", "DDDDAAAA"[:NPAIR])
LSLICE = 512
NSL = (S * S) // LSLICE   # 8 logits slices per tile
GCHUNK = NCORES * NSUB    # 32 global chunks
NRN = NCORES              # combine renorm slots (renorm every u-chain step)

OFF_Q = 0                            # B per-core operators, 64x64 bf16 each
OFF_A0B = OFF_Q + B * S * S // 2         # [B, S] bf16 a0
OFF_NU = OFF_A0B + B * S // 2            # [B] f32
CONTRIB = OFF_NU + B


def build_program():
    nc = bacc.Bacc(
        "TRN2",
        target_bir_lowering=False,
        debug=False,
        enable_asserts=False,
        num_devices=NCORES,
    )

    def din(name, shape, dtype=F32):
        return nc.dram_tensor(name, list(shape), dtype, kind="ExternalInput").ap()

    io = {}
    io["obs"] = din("obs_slab", (B, SLAB, D))
    io["ctx"] = din("ctx_slab", (B, SLAB, C), BF16)
    io["alpha"] = din("alpha_pair", (128, NPAIR))
    io["tw1cT"] = din("tw1cT", (C, H), BF16)
    io["tb1p"] = din("tb1p", (H,))
    io["tw2T"] = din("tw2T", (H, H), BF16)
    io["tb2"] = din("tb2", (H,))
    io["tw3T"] = din("tw3T", (H, S * S), BF16)
    io["tb3"] = din("tb3_bf", (S * S,), BF16)
    io["fw1T"] = din("fw1T", (D, H), BF16)
    io["fb1"] = din("fb1_", (H,))
    io["fw2T"] = din("fw2T", (H, H), BF16)
    io["fb2"] = din("fb2_", (H,))
    io["mwT"] = din("mwT", (H, D), BF16)
    io["mb"] = din("mb_", (D,))
    io["lwT"] = din("lwT", (H, D), BF16)
    io["lb"] = din("lb_", (D,))
    io["lwsum"] = din("lwsum", (H,), BF16)
    io["L1"] = din("L1", (D, S), BF16)
    io["L2m"] = din("L2m", (D, S), BF16)
    io["L3"] = din("L3", (D, S), BF16)
    io["olv_bias"] = din("olv_bias", (S,))

    io["out"] = nc.dram_tensor("ll_out", [B], F32, kind="ExternalOutput").ap()
    io["Ebuf"] = nc.dram_tensor("Ebuf", [B, SLAB, S * S], BF16).ap()
    io["initd"] = nc.dram_tensor("initd", [B, NSUB, S], F32).ap()
    io["vd"] = nc.dram_tensor("vd", [B, S], F32).ap()
    io["lnd"] = nc.dram_tensor("lnd_bounce", [B, B], F32).ap()
    io["Mlocal"] = nc.dram_tensor("Mlocal", [NCHAIN, S, S], BF16).ap()
    io["contrib"] = nc.dram_tensor("contrib", [CONTRIB], F32).ap()
    io["gathered"] = nc.dram_tensor(
        "gathered", [NCORES * CONTRIB], F32, addr_space="Shared"
    ).ap()

    with tile.TileContext(nc) as tc:
        build_tile(tc, io)
    nc.compile()
    return nc


def build_tile(tc, io):
    nc = tc.nc
    dma = nc.sync
    with (
        tc.tile_pool(name="wts", bufs=1) as wts,
        tc.tile_pool(name="consts", bufs=1) as consts,
    ):
        ident = consts.tile([128, 128], F32)
        make_identity(nc, ident)
        identb = consts.tile([128, 128], BF16)
        nc.vector.tensor_copy(identb, ident)
        # stacked double identity [128, 64]: rows k, col k%64
        i2b = consts.tile([128, S], BF16)
        nc.vector.tensor_add(i2b, identb[:, 0:S], identb[:, S:2 * S])

        def load_w(tag, ap_dram, shape, dtype=F32):
            t = wts.tile(list(shape), dtype, tag=tag)
            dma.dma_start(out=t, in_=ap_dram)
            return t

        w = {}
        w["tw1cT"] = load_w("w1", io["tw1cT"], (C, H), BF16)
        w["tw2T"] = [load_w(f"w2{k}", io["tw2T"][k * 128:(k + 1) * 128, :],
                            (128, H), BF16) for k in range(2)]
        w["tw3T"] = [load_w(f"w3{k}", io["tw3T"][k * 128:(k + 1) * 128, :],
                            (128, S * S), BF16) for k in range(2)]
        w["tb3"] = load_w("b3", io["tb3"][None, :], (1, S * S), BF16)
        w["fw1T"] = load_w("g1", io["fw1T"], (D, H), BF16)
        w["fw2T"] = [load_w(f"g2{k}", io["fw2T"][k * 128:(k + 1) * 128, :],
                            (128, H), BF16) for k in range(2)]
        w["mwT"] = [load_w(f"gm{k}", io["mwT"][k * 128:(k + 1) * 128, :],
                           (128, D), BF16) for k in range(2)]
        w["lwT"] = [load_w(f"gl{k}", io["lwT"][k * 128:(k + 1) * 128, :],
                           (128, D), BF16) for k in range(2)]
        w["lwsum"] = [load_w(f"ls{k}", io["lwsum"][k * 128:(k + 1) * 128, None],
                             (128, 1), BF16) for k in range(2)]
        w["L1"] = load_w("L1", io["L1"], (D, S), BF16)
        w["L2m"] = load_w("L2m", io["L2m"], (D, S), BF16)
        w["L3"] = load_w("L3", io["L3"], (D, S), BF16)
        for nm in ("tb1p", "tb2", "fb1", "fb2"):
            w[nm] = [load_w(f"{nm}{k}", io[nm][k * 128:(k + 1) * 128, None],
                            (128, 1)) for k in range(2)]
        w["mb"] = load_w("mb", io["mb"][:, None], (D, 1))
        w["lb"] = load_w("lb", io["lb"][:, None], (D, 1))
        w["olvb"] = load_w("olvb", io["olv_bias"][:, None], (S, 1))
        w["alpha"] = load_w("alpha", io["alpha"], (128, NPAIR))

        ones_bt = consts.tile([1, NT], BF16)        # lhsT for tb3 rank-1
        nc.vector.memset(ones_bt, 1.0)
        pones_bf = consts.tile([1, S], BF16)        # +1 lhsT (sum_blv bcast)
        nc.vector.memset(pones_bf, 1.0)
        pones128 = consts.tile([1, 128], F32)       # rbc broadcast (combine)
        nc.vector.memset(pones128, 1.0)
        nones_row = consts.tile([1, S], F32)        # -1 lhsT (mean bcast)
        nc.vector.memset(nones_row, -1.0)
        invS_col = consts.tile([S, 1], F32)
        nc.vector.memset(invS_col, 1.0 / S)
        ones128 = consts.tile([128, 1], BF16)       # mass lhsT (combine)
        nc.vector.memset(ones128, 1.0)
        w.update(ones_bt=ones_bt, pones_bf=pones_bf, pones128=pones128,
                 nones_row=nones_row, invS_col=invS_col, ones128=ones128,
                 ident=ident, identb=identb, i2b=i2b)

        nu_sb = consts.tile([1, B], F32)
        a0_sb = consts.tile([S, B], F32)
        sp0_sb = consts.tile([S, B], F32)
        initv_sb = consts.tile([S, B, NSUB], F32)
        nslots_sb = consts.tile([1, B, NSUB], F32)

        # eseg tiles live across both phases; zero the off-diag blocks once,
        # before phase 1, so the memsets overlap compute
        with tc.tile_pool(name="p2e", bufs=1) as p2e:
            eseg = []
            for p in range(NPAIR):
                row = []
                for s in range(2):
                    et = p2e.tile([128, SEG, 128], BF16, tag=f"es{p}_{s}")
                    nc.gpsimd.memset(et, 0.0)
                    row.append(et)
                eseg.append(row)
            for rep in range(REPEAT):
                build_main(tc, io, w, dma, nu_sb, a0_sb, sp0_sb, initv_sb,
                           nslots_sb, eseg, rep)


def build_main(tc, io, w, dma, nu_sb, a0_sb, sp0_sb, initv_sb, nslots_sb,
               eseg, rep=0):
    nc = tc.nc
    contrib = io["contrib"]
    if True:
        # ================= PHASE 1 =================
        with (
            tc.tile_pool(name="p1", bufs=3) as p1,
            tc.tile_pool(name="p1es", bufs=2) as p1es,
            tc.tile_pool(name="pbig", bufs=3, space="PSUM") as pbig,
            tc.tile_pool(name="plg", bufs=2, space="PSUM") as plg,
            tc.tile_pool(name="psm", bufs=3, space="PSUM") as psm,
        ):
            if P2ONLY:
                for t in (a0_sb, sp0_sb, initv_sb, nslots_sb):
                    nc.vector.memset(t, 1.0)
            for b in range(B):
                for hh in range(NTILES):
                    if not P2ONLY:
                        phase1_tile(nc, b, hh, dma, p1, p1es, pbig, plg, psm,
                                    w, io, a0_sb, sp0_sb, initv_sb, nslots_sb)
            for b in range(B):
                nc.vector.reduce_sum(nu_sb[:, b:b + 1], nslots_sb[:, b, :],
                                     axis=AX.X)
            # small DRAM bounces for phase-2 init (cross-partition placement)
            dma.dma_start(
                out=io["initd"].rearrange("b n s -> s (b n)"),
                in_=initv_sb.rearrange("s b n -> s (b n)"),
            )
            vtmp = p1.tile([S, B], F32, tag="vtmp")
            nc.vector.reciprocal(vtmp, a0_sb)
            vtmp2 = p1.tile([S, B], F32, tag="vtmp2")
            nc.vector.tensor_mul(vtmp2, vtmp, sp0_sb)
            dma.dma_start(out=io["vd"].rearrange("b s -> s b"), in_=vtmp2)
            contrib = io["contrib"]
            dma.dma_start(out=contrib[OFF_NU:OFF_NU + B][None, :], in_=nu_sb)
            a0bf = p1.tile([S, B], BF16, tag="a0bf")
            nc.vector.tensor_copy(a0bf, a0_sb)
            dma.dma_start(
                out=contrib[OFF_A0B:OFF_A0B + B * S // 2].bitcast(BF16)
                .rearrange("(b s2) -> s2 b", s2=S),
                in_=a0bf,
            )

        if BUILD_STAGE < 2:
            return
        # ================= PHASE 2 =================
        with (
            tc.tile_pool(name="p2s", bufs=2) as p2s,
            tc.tile_pool(name="p2x", bufs=2) as p2x,
            tc.tile_pool(name="p2ps", bufs=1, space="PSUM") as p2ps,
        ):
            xfin = run_chains(nc, dma, eseg, p2s, p2x, p2ps, w, io)
            # chunk operators -> local DRAM (bounce to get all blocks onto
            # partitions 0:64 for the local pre-combine)
            for p in range(NPAIR):
                for h in range(2):
                    c = p * 2 + h
                    dma.dma_start(
                        out=io["Mlocal"][c],
                        in_=xfin[p][h * S:(h + 1) * S, :],
                    )
            # local pre-combine: per b, multiply this core's 4 chunk
            # operators (ascending sub) into one; ship only those 4.
            mload = p2s.tile([S, NCHAIN, S], BF16, tag="mload")
            dma.dma_start(out=mload,
                          in_=io["Mlocal"].rearrange("c a m -> a c m"))

            def prodmm(left, right, pj):
                # left @ right via lhsT = left^T (PE transpose, base 0)
                tps = p2ps.tile([S, S], BF16, tag=f"ps{pj}")
                nc.tensor.transpose(tps, left, w["identb"][:S, :S])
                tsb = p2x.tile([S, S], BF16, tag=f"tsb{pj}")
                nc.vector.tensor_copy(tsb, tps)
                pps = p2ps.tile([S, S], F32, tag=f"ps{pj + 1}")
                nc.tensor.matmul(pps, tsb, right)
                osb = p2x.tile([S, S], BF16, tag=f"osb{pj}")
                nc.vector.tensor_copy(osb, pps)
                return osb

            qreg = contrib[OFF_Q:OFF_Q + B * S * S // 2].bitcast(BF16)
            for b in range(B):
                bp, h = b // 2, b % 2

                def opv(sub):
                    return mload[:, bp * 2 * NSUB + sub * 2 + h, :]

                p1r = prodmm(opv(1), opv(0), (2 * b) % 6)
                p2r = prodmm(opv(3), opv(2), (2 * b + 1) % 6)
                qb = prodmm(p2r, p1r, (2 * b) % 6)
                dma.dma_start(
                    out=qreg[b * S * S:(b + 1) * S * S]
                    .rearrange("(a m) -> a m", a=S),
                    in_=qb,
                )
        if BUILD_STAGE < 3:
            return
        if not NOCC:
            with tc.tile_critical():
                with nc.semaphore(f"ccsem{rep}") as ccsem:
                    nc.gpsimd.collective_compute(
                        "AllGather",
                        ALU.bypass,
                        replica_groups=[list(range(NCORES))],
                        ins=[contrib],
                        outs=[io["gathered"]],
                    ).then_inc(ccsem, 1)
                    nc.gpsimd.wait_ge(ccsem, 1)

        # ================= COMBINE =================
        with (
            tc.tile_pool(name="cmb", bufs=2) as cmb,
            tc.tile_pool(name="cmbps", bufs=1, space="PSUM") as cmbps,
        ):
            combine(nc, dma, cmb, cmbps, w, io)


def phase1_tile(nc, b, hh, dma, p1, p1es, pbig, plg, psm, w, io,
                a0_sb, sp0_sb, initv_sb, nslots_sb):
    nt = NT
    t0 = hh * NT
    ident = w["ident"]

    ctx_t = p1.tile([nt, C], BF16, tag="ctxt")
    dma.dma_start(out=ctx_t, in_=io["ctx"][b, t0:t0 + nt, :])
    obs_t = p1.tile([nt, D], F32, tag="obst")
    dma.dma_start(out=obs_t, in_=io["obs"][b, t0:t0 + nt, :])

    ctxT_ps = pbig.tile([C, nt], BF16, tag="pp")
    nc.tensor.transpose(ctxT_ps, ctx_t, w["identb"])
    ctxT = p1.tile([C, nt], BF16, tag="ctxT")
    nc.vector.tensor_copy(ctxT, ctxT_ps)

    obsT_ps = psm.tile([D, nt], F32, tag="sm")
    nc.tensor.transpose(obsT_ps, obs_t, ident)
    obsT = p1.tile([D, nt], F32, tag="obsT")
    nc.vector.tensor_copy(obsT, obsT_ps)
    obsT_bf = p1.tile([D, nt], BF16, tag="obsTb")
    nc.scalar.copy(obsT_bf, obsT_ps)

    # transition MLP (feature-on-partition, bf16)
    h1 = []
    for m in range(2):
        ps = pbig.tile([128, nt], F32, tag="pp")
        nc.tensor.matmul(ps, w["tw1cT"][:, m * 128:(m + 1) * 128], ctxT)
        sb = p1.tile([128, nt], BF16, tag=f"h1_{m}")
        nc.scalar.activation(sb, ps, AF.Relu, bias=w["tb1p"][m], scale=1.0)
        h1.append(sb)
    h2 = []
    for m in range(2):
        ps = pbig.tile([128, nt], F32, tag="pp")
        for k in range(2):
            nc.tensor.matmul(ps, w["tw2T"][k][:, m * 128:(m + 1) * 128], h1[k],
                             start=(k == 0), stop=(k == 1))
        sb = p1.tile([128, nt], BF16, tag=f"h2_{m}")
        nc.scalar.activation(sb, ps, AF.Relu, bias=w["tb2"][m], scale=1.0)
        h2.append(sb)

    # logits slices -> exp (bf16) into the big E tile; R accumulation
    E_all = p1es.tile([nt, S * S], BF16, tag="eall")
    R_sb = p1.tile([nt, S], F32, tag="Rsb")
    ni = LSLICE // S
    for sl in range(NSL):
        ps = plg.tile([nt, LSLICE], F32, tag="lgp")
        for k in range(2):
            nc.tensor.matmul(ps, h2[k],
                             w["tw3T"][k][:, sl * LSLICE:(sl + 1) * LSLICE],
                             start=(k == 0), stop=False)
        nc.tensor.matmul(ps, w["ones_bt"],
                         w["tb3"][:, sl * LSLICE:(sl + 1) * LSLICE],
                         start=False, stop=True)
        esl = E_all[:, sl * LSLICE:(sl + 1) * LSLICE]
        nc.scalar.activation(esl, ps, AF.Exp)
        nc.vector.reduce_sum(
            R_sb[:, sl * ni:(sl + 1) * ni],
            esl.rearrange("p (i j) -> p i j", j=S),
            axis=AX.X,
        )

    # observation model (bf16 MLP)
    f1 = []
    for m in range(2):
        ps = pbig.tile([128, nt], F32, tag="pp")
        nc.tensor.matmul(ps, w["fw1T"][:, m * 128:(m + 1) * 128], obsT_bf)
        sb = p1.tile([128, nt], BF16, tag=f"f1_{m}")
        nc.scalar.activation(sb, ps, AF.Relu, bias=w["fb1"][m], scale=1.0)
        f1.append(sb)
    f2 = []
    for m in range(2):
        ps = pbig.tile([128, nt], F32, tag="pp")
        for k in range(2):
            nc.tensor.matmul(ps, w["fw2T"][k][:, m * 128:(m + 1) * 128], f1[k],
                             start=(k == 0), stop=(k == 1))
        sb = p1.tile([128, nt], BF16, tag=f"f2_{m}")
        nc.scalar.activation(sb, ps, AF.Relu, bias=w["fb2"][m], scale=1.0)
        f2.append(sb)

    bm_ps = psm.tile([D, nt], F32, tag="sm")
    for k in range(2):
        nc.tensor.matmul(bm_ps, w["mwT"][k], f2[k], start=(k == 0), stop=(k == 1))
    bm = p1.tile([D, nt], F32, tag="bm")
    nc.scalar.activation(bm, bm_ps, AF.Identity, bias=w["mb"], scale=1.0)

    blv_ps = psm.tile([D, nt], F32, tag="sm")
    for k in range(2):
        nc.tensor.matmul(blv_ps, w["lwT"][k], f2[k], start=(k == 0), stop=(k == 1))
    blv = p1.tile([D, nt], F32, tag="blv")
    nc.scalar.activation(blv, blv_ps, AF.Identity, bias=w["lb"], scale=1.0)

    r_ = p1.tile([D, nt], F32, tag="r_")
    nc.vector.tensor_sub(r_, obsT, bm)
    e_ = p1.tile([D, nt], F32, tag="e_")
    nc.scalar.activation(e_, blv, AF.Exp, scale=-1.0)
    rr = p1.tile([D, nt], F32, tag="rr")
    nc.vector.tensor_mul(rr, r_, r_)
    A_ = p1.tile([D, nt], BF16, tag="A_")
    nc.vector.tensor_mul(A_, rr, e_)
    Bm_ = p1.tile([D, nt], BF16, tag="Bm_")
    nc.vector.tensor_mul(Bm_, r_, e_)
    e_bf = p1.tile([D, nt], BF16, tag="ebf")
    nc.vector.tensor_copy(e_bf, e_)

    sb_ps = psm.tile([1, nt], F32, tag="sm")
    for k in range(2):
        nc.tensor.matmul(sb_ps, w["lwsum"][k], f2[k], start=(k == 0),
                         stop=(k == 1))
    sblv = p1.tile([1, nt], BF16, tag="sblv")
    nc.scalar.copy(sblv, sb_ps)

    q_ps = psm.tile([S, nt], F32, tag="sm")
    nc.tensor.matmul(q_ps, w["L1"], A_, start=True, stop=False)
    nc.tensor.matmul(q_ps, w["L2m"], Bm_, start=False, stop=False)
    nc.tensor.matmul(q_ps, w["L3"], e_bf, start=False, stop=False)
    nc.tensor.matmul(q_ps, w["pones_bf"], sblv, start=False, stop=True)

    lp0 = p1.tile([S, nt], F32, tag="lp0")
    nc.scalar.activation(lp0, q_ps, AF.Identity, bias=w["olvb"], scale=-0.5)

    n_ps = psm.tile([1, nt], F32, tag="sm")
    nc.tensor.matmul(n_ps, w["invS_col"], lp0)
    n_sb = p1.tile([1, nt], F32, tag="nsb")
    nc.scalar.copy(n_sb, n_ps)
    d_ps = psm.tile([S, nt], F32, tag="sm")
    nc.tensor.matmul(d_ps, w["nones_row"], n_sb)      # = -n broadcast
    dd = p1.tile([S, nt], F32, tag="dd")
    nc.vector.tensor_add(dd, lp0, d_ps)
    ehat = p1.tile([S, nt], F32, tag="ehat")
    nc.scalar.activation(ehat, dd, AF.Exp)

    # invR and the per-step scale s (transposed form first)
    RT_ps = psm.tile([S, nt], F32, tag="sm")
    nc.tensor.transpose(RT_ps, R_sb, ident)
    invR = p1.tile([S, nt], F32, tag="invR")
    nc.vector.reciprocal(invR, RT_ps)

    shT = p1.tile([S, nt], F32, tag="shT")
    for cc in range(nt // CHUNK):
        lo = cc * CHUNK
        nc.vector.tensor_mul(shT[:, lo:lo + CHUNK - 1],
                             ehat[:, lo:lo + CHUNK - 1],
                             invR[:, lo + 1:lo + CHUNK])
        nc.vector.tensor_copy(shT[:, lo + CHUNK - 1:lo + CHUNK],
                              ehat[:, lo + CHUNK - 1:lo + CHUNK])

    sh_ps = psm.tile([nt, S], F32, tag="sm")
    nc.tensor.transpose(sh_ps, shT, ident[:S, :S])
    shat = p1.tile([nt, S], BF16, tag="shat")
    nc.vector.tensor_copy(shat, sh_ps)

    # pre-scale the big E tile along j, then single DMA out. SBUF-only
    # elementwise, so part of it can ride the (otherwise idle) GPSIMD.
    ev3 = E_all.rearrange("p (i j) -> p i j", j=S)
    shb = shat[:, None, :]
    psplit = os.environ.get("NHMM_PSPLIT", "DDPP")
    for q4 in range(4):
        blk = 16
        eng = nc.gpsimd if psplit[q4] == "P" else nc.vector
        eng.tensor_mul(
            ev3[:, q4 * blk:(q4 + 1) * blk, :],
            ev3[:, q4 * blk:(q4 + 1) * blk, :],
            shb.broadcast_to([nt, blk, S]),
        )
    ebq = os.environ.get("NHMM_EBUFQ", "pool")
    eng = {"sp": nc.sync, "act": nc.scalar, "pool": nc.gpsimd}[ebq]
    eng.dma_start(out=io["Ebuf"][b, t0:t0 + nt, :], in_=E_all)

    # persists for phase 2 / combine
    for cc in range(nt // CHUNK):
        sub = hh * (nt // CHUNK) + cc
        nc.vector.tensor_copy(initv_sb[:, b, sub:sub + 1],
                              invR[:, cc * CHUNK:cc * CHUNK + 1])
    if hh == 0:
        nc.vector.tensor_copy(a0_sb[:, b:b + 1], ehat[:, 0:1])
        nc.vector.tensor_copy(sp0_sb[:, b:b + 1], shT[:, 0:1])
    nc.vector.reduce_sum(nslots_sb[:, b, hh:hh + 1], n_sb, axis=AX.X)


def run_chains(nc, dma, eseg, p2s, p2x, p2ps, w, io):
    """8 pair-chains; pair p = (bpair=p//NSUB, sub=p%NSUB) packs chains
    b=2*bpair and b=2*bpair+1 block-diagonally."""
    alpha = w["alpha"]
    Ebuf = io["Ebuf"]
    ev = [Ebuf[b].rearrange("t (i j) -> i t j", i=S) for b in range(B)]

    def issue_seg(p, sgi):
        bp, sub = p // NSUB, p % NSUB
        lt0 = sub * CHUNK + sgi * SEG
        et = eseg[p][sgi % 2]
        for h in range(2):
            b = 2 * bp + h
            dma.dma_start(
                out=et[h * S:(h + 1) * S, :, h * S:(h + 1) * S],
                in_=ev[b][:, lt0:lt0 + SEG, :],
            )

    for p in range(NPAIR):
        issue_seg(p, 0)
        issue_seg(p, 1)

    # X init: stacked diag(1 + alpha*(invR_lo - 1))
    xall = []
    for p in range(NPAIR):
        bp, sub = p // NSUB, p % NSUB
        irp = p2s.tile([128, 1], F32, tag=f"irp{p}")
        for h in range(2):
            nc.scalar.dma_start(
                out=irp[h * S:(h + 1) * S, :],
                in_=io["initd"][2 * bp + h, sub, :][:, None],
            )
        t1 = p2s.tile([128, 1], F32, tag="t1")
        nc.vector.tensor_scalar_add(t1, irp, -1.0)
        t2 = p2s.tile([128, 1], F32, tag="t2")
        nc.vector.tensor_mul(t2, t1, alpha[:, p:p + 1])
        t3 = p2s.tile([128, 1], F32, tag="t3")
        nc.vector.tensor_scalar_add(t3, t2, 1.0)
        xp = p2x.tile([128, S], BF16, tag=f"x{p}")
        nc.vector.tensor_scalar_mul(xp, w["i2b"], t3)
        xall.append(xp)

    # step-0 blend for sub=0 pairs: et0 <- alpha*et0 + (1-alpha)*D(v)
    for p in range(NPAIR):
        if p % NSUB != 0:
            continue
        bp = p // NSUB
        vp = p2s.tile([128, 1], F32, tag=f"vp{p}")
        for h in range(2):
            nc.scalar.dma_start(
                out=vp[h * S:(h + 1) * S, :],
                in_=io["vd"][2 * bp + h, :][:, None],
            )
        w1 = p2s.tile([128, 1], F32, tag="w1")
        nc.vector.tensor_scalar(w1, alpha[:, p:p + 1], -1.0, 1.0,
                                op0=ALU.mult, op1=ALU.add)
        w2 = p2s.tile([128, 1], F32, tag="w2")
        nc.vector.tensor_mul(w2, w1, vp)
        dmat = p2s.tile([128, 128], BF16, tag="dmat")
        nc.vector.tensor_scalar_mul(dmat, w["identb"], w2)
        et0 = eseg[p][0][:, 0, :]
        tmp_e = p2s.tile([128, 128], BF16, tag="tmpe")
        nc.vector.tensor_scalar_mul(tmp_e, et0, alpha[:, p:p + 1])
        nc.vector.tensor_add(et0, tmp_e, dmat)

    # copy-engine split per NHMM_CMAP (D=DVE, A=Act)
    def copy_x(p, new_x, ps):
        if CMAP[p % len(CMAP)] == "D":
            nc.vector.tensor_copy(new_x, ps)
        else:
            nc.scalar.copy(new_x, ps)

    for k in range(CHUNK):
        sgi, tt = k // SEG, k % SEG
        pss = []
        for p in range(NPAIR):
            ps = p2ps.tile([128, S], F32, tag=f"ps{p}")
            nc.tensor.matmul(ps, eseg[p][sgi % 2][:, tt, :], xall[p])
            pss.append(ps)
        for p in range(NPAIR):
            new_x = p2x.tile([128, S], BF16, tag=f"x{p}")
            copy_x(p, new_x, pss[p])
            xall[p] = new_x
        if tt == SEG - 1 and sgi + 2 < NSEG:
            for p in range(NPAIR):
                issue_seg(p, sgi + 2)

    return xall


def combine(nc, dma, cmb, cmbps, w, io):
    if NOCC:
        def gsl(core, lo, n):
            return io["contrib"][lo:lo + n]
    else:
        g2 = io["gathered"].rearrange("(k f) -> k f", k=NCORES)

        def gsl(core, lo, n):
            return g2[core, lo:lo + n]

    # Bulk-load the per-core pre-combined operators: one [64, B, 64] tile
    # per core (one DMA each); everything sits on partitions 0:64.
    mstore = {}
    for core in range(NCORES - 1, -1, -1):
        mv = (gsl(core, OFF_Q, B * S * S // 2).bitcast(BF16)
              .rearrange("(b a m) -> a b m", b=B, a=S))
        mt = cmb.tile([S, B, S], BF16, tag=f"ms_{core}")
        eng = dma if core % 2 == 0 else nc.scalar
        eng.dma_start(out=mt, in_=mv)
        mstore[core] = mt

    # u: [64, B] — one column per batch element
    u = cmb.tile([S, B], BF16, tag="u")
    nc.vector.memset(u, 1.0)
    mslots = cmb.tile([1, B, NRN], F32, tag="mslots")
    nc.vector.memset(mslots, 1.0)

    for step, core in enumerate(range(NCORES - 1, -1, -1)):
        mt = mstore[core]
        up = cmbps.tile([S, B], F32, tag="up")
        for b in range(B):
            nc.tensor.matmul(up[:, b:b + 1], mt[:, b, :], u[:, b:b + 1])
        un = cmb.tile([S, B], BF16, tag="u")
        nc.vector.tensor_copy(un, up)
        u = un
        # renorm every step: per-core operators span 256 timesteps, so
        # column masses can reach ~1e12 per application
        ms = cmbps.tile([1, B], F32, tag="msp")
        nc.tensor.matmul(ms, w["ones128"][0:S, :], un)
        nc.scalar.copy(mslots[:, :, step], ms)
        minv = cmb.tile([1, B], F32, tag="mi")
        nc.vector.reciprocal(minv, ms)
        rbc = cmbps.tile([S, B], F32, tag="rb")
        nc.tensor.matmul(rbc, w["pones128"][:, 0:S], minv)
        u2 = cmb.tile([S, B], BF16, tag="u")
        nc.vector.tensor_mul(u2, un, rbc)
        u = u2

    # dots with a0, logs, nu sums
    a0p = cmb.tile([S, B], BF16, tag="a0p")
    a0bit = gsl(0, OFF_A0B, B * S // 2).bitcast(BF16)
    for b in range(B):
        dma.dma_start(out=a0p[:, b:b + 1],
                      in_=a0bit[b * S:(b + 1) * S][:, None])
    # per-b dots as [1,1] matmuls: every result lands on partition 0, so no
    # cross-partition bounce is needed for the final assembly
    dot = cmbps.tile([1, B], F32, tag="dot")
    for b in range(B):
        nc.tensor.matmul(dot[:, b:b + 1], u[:, b:b + 1], a0p[:, b:b + 1])
    lnrow = cmb.tile([1, B], F32, tag="lnrow")
    nc.scalar.activation(lnrow, dot, AF.Ln)

    lnms = cmb.tile([1, B, NRN], F32, tag="lnms")
    nc.scalar.activation(lnms, mslots, AF.Ln)
    lnm_sb = cmb.tile([1, B], F32, tag="lnm")
    for b in range(B):
        nc.vector.reduce_sum(lnm_sb[:, b:b + 1], lnms[:, b, :], axis=AX.X)

    nurow = cmb.tile([1, B, NCORES], F32, tag="nur")
    for b in range(B):
        if NOCC:
            for kk in range(NCORES):
                dma.dma_start(out=nurow[:, b, kk:kk + 1],
                              in_=io["contrib"][OFF_NU + b:OFF_NU + b + 1][None, :])
        else:
            g2 = io["gathered"].rearrange("(k f) -> k f", k=NCORES)
            dma.dma_start(out=nurow[:, b, :], in_=g2[:, OFF_NU + b][None, :])
    nusum = cmb.tile([1, B], F32, tag="nus")
    for b in range(B):
        nc.vector.reduce_sum(nusum[:, b:b + 1], nurow[:, b, :], axis=AX.X)

    acc1 = cmb.tile([1, B], F32, tag="acc1")
    nc.vector.tensor_add(acc1, lnrow, lnm_sb)
    acc2 = cmb.tile([1, B], F32, tag="acc2")
    nc.vector.tensor_add(acc2, acc1, nusum)
    out_row = cmb.tile([1, B], F32, tag="outrow")
    nc.vector.tensor_scalar_add(out_row, acc2, -math.log(S))
    dma.dma_start(out=io["out"][None, :], in_=out_row)


# ======================================================================
# host side
# ======================================================================
_PROGRAM_CACHE = {}


def _get_program():
    if "nc" not in _PROGRAM_CACHE:
        _PROGRAM_CACHE["nc"] = build_program()
    return _PROGRAM_CACHE["nc"]


def host_prep(inp):
    f32 = np.float32
    bf = ml_dtypes.bfloat16
    p = {}
    tw1 = np.asarray(inp["tw1"], f32)
    p["tw1cT"] = np.ascontiguousarray(tw1[:, :C].T).astype(bf)
    p["tb1p"] = (np.asarray(inp["tb1"], f32) + tw1[:, C:].sum(1) / S).astype(f32)
    p["tw2T"] = np.ascontiguousarray(np.asarray(inp["tw2"], f32).T).astype(bf)
    p["tb2"] = np.asarray(inp["tb2"], f32)
    p["tw3T"] = np.ascontiguousarray(np.asarray(inp["tw3"], f32).T).astype(bf)
    p["tb3_bf"] = np.asarray(inp["tb3"], f32).astype(bf)
    p["fw1T"] = np.ascontiguousarray(np.asarray(inp["fw1"], f32).T).astype(bf)
    p["fb1_"] = np.asarray(inp["fb1"], f32)
    p["fw2T"] = np.ascontiguousarray(np.asarray(inp["fw2"], f32).T).astype(bf)
    p["fb2_"] = np.asarray(inp["fb2"], f32)
    lw = np.asarray(inp["lw"], f32)
    p["mwT"] = np.ascontiguousarray(np.asarray(inp["mw"], f32).T).astype(bf)
    p["mb_"] = np.asarray(inp["mb"], f32)
    p["lwT"] = np.ascontiguousarray(lw.T).astype(bf)
    p["lb_"] = np.asarray(inp["lb"], f32)
    p["lwsum"] = lw.sum(0).astype(bf)
    se = np.asarray(inp["state_emb"], f32)
    off_mean = se @ np.asarray(inp["mw"], f32).T
    off_lv = se @ lw.T
    E1 = np.exp(-off_lv)
    p["L1"] = np.ascontiguousarray(E1.T).astype(bf)
    p["L2m"] = np.ascontiguousarray((-2.0 * off_mean * E1).T).astype(bf)
    p["L3"] = np.ascontiguousarray((off_mean**2 * E1).T).astype(bf)
    p["olv_bias"] = (
        -0.5 * (D * math.log(2.0 * math.pi) + off_lv.sum(1))
        - 0.5 * np.asarray(inp["lb"], f32).sum()
    ).astype(f32)
    return p


def make_in_maps(inp, prepped=None):
    p = prepped if prepped is not None else host_prep(inp)
    obs = np.asarray(inp["observations"], np.float32)
    ctx = np.asarray(inp["context"], np.float32).astype(ml_dtypes.bfloat16)
    in_maps = []
    for k in range(NCORES):
        t0, t1 = SLAB * k, SLAB * (k + 1)
        alpha = np.ones((128, NPAIR), np.float32)
        if k == 0:
            alpha[:, 0] = 0.0   # sub-0 pairs: global t=0 correction
            alpha[:, NSUB] = 0.0
        m = {
            "obs_slab": np.ascontiguousarray(obs[:, t0:t1, :]),
            "ctx_slab": np.ascontiguousarray(ctx[:, t0:t1, :]),
            "alpha_pair": alpha,
        }
        m.update(p)
        in_maps.append(m)
    return in_maps


def kernel(**inputs):
    nc = _get_program()
    in_maps = make_in_maps(inputs)
    res = run_bass_kernel_spmd(nc, in_maps, core_ids=list(range(NCORES)))
    return np.asarray(res.results[0]["ll_out"], np.float32)


if __name__ == "__main__":
    sys.path.insert(0, "/root/problem")
    import jax

    with jax.default_device(jax.devices("cpu")[0]):
        import reference

        inp = {k: np.asarray(v) for k, v in reference.setup_inputs().items()}
        expected = np.asarray(reference.reference(**inp))
    got = kernel(**inp)
    print("expected:", expected)
    print("kernel:  ", got)
    rel = np.abs(got - expected) / np.abs(expected)
    print(f"rel: {rel.max():.3e}")
